# revision 12
# baseline (speedup 1.0000x reference)
"""DeepFFM Trainium2 kernel (8 NeuronCores, SPMD via bass/Tile) — v2.

Math (reference):
  linear      = X @ w1 + b
  S[i,j]      = <nfk[i, f2f[j], :], nfk[j, f2f[i], :]>   (symmetric, param-only)
  interaction = 0.5 * (x^T S x - sum_i S_ii x_i^2)  per batch row
  deep        = MLP(X) with relu layers
  out         = linear + interaction + deep

Strategy (v2 — no S AllGather):
  * Features are host-sorted by field. The 39 field groups are bin-packed
    into 8 bins of <=128 rows and <=NSLOT groups; core c builds ONLY the
    S rows of its bin, as ST[j, i] = S[i, j] tiles [125 x 128] via
    stacked-contraction matmuls: 3 groups share one 120-partition
    contraction, with zero-filled slots in the host-built R tensor
    encoding per-core group boundaries (SPMD-uniform program, per-core
    data).
  * Interaction partial: Y = ST^T @ XT over the FULL 4096 batch
    (XT loaded whole, 8.2MB), z_c[b] = 0.5*sum_i x_ib*Y_ib - 0.5*d_i x_ib^2
    (d = diag(S), host-computed from nfk). The per-core z partials
    [1,4096] fp32 are combined with a 16KB ReduceScatter (the fabric
    runs ~50GB/s — a 2.5MB S AllGather costs ~55us, 16KB is latency
    only and hides under the deep MLP).
  * Deep MLP + linear are batch-sharded (512 rows/core) exactly as v1.
  * All inputs are host-prearranged to exact SBUF layouts; bf16
    throughout with fp32 PSUM accumulation.
"""

import numpy as np

import concourse.bass as bass
import concourse.bacc as bacc
import concourse.mybir as mybir
import concourse.tile as tile
from concourse.bass_utils import run_bass_kernel_spmd

F32 = mybir.dt.float32
BF16 = mybir.dt.bfloat16

NCORES = 8
B = 4096
BS = B // NCORES          # batch rows per core
F = 1000                  # feature size
FIELDS = 39
K = 40                    # ffm embedding dim
D0, D1, D2 = 1024, 512, 256
KT0 = 8                   # k-chunks (125) over F
P_F = 125                 # partition chunk of F


def _pack_bins(counts, nslot):
    """Deterministic bin packing: 8 bins, <=128 rows, <=nslot groups."""
    import random
    rng = random.Random(0)
    base = list(np.argsort(-counts))

    def attempt(order):
        bins = [[] for _ in range(NCORES)]
        rows = [0] * NCORES
        for g in order:
            cand = [(128 - (rows[bb] + counts[g]), bb) for bb in range(NCORES)
                    if rows[bb] + counts[g] <= 128 and len(bins[bb]) < nslot]
            if not cand:
                return None
            _, bb = min(cand)
            bins[bb].append(int(g))
            rows[bb] += int(counts[g])
        return bins

    for trial in range(50000):
        order = base[:]
        if trial:
            rng.shuffle(order)
            order.sort(key=lambda g: -counts[g] + rng.uniform(-6, 6))
        bins = attempt(order)
        if bins:
            return bins
    return None


def _build_program(fsegs, nstack, bias_const, sim_single_core=False):
    """fsegs: list of (f, j0, j1) — global sorted-feature col range of each
    field (compile-time, baked into the program)."""
    from concourse.masks import make_identity
    nc = bacc.Bacc(None, num_devices=NCORES)

    xtf_h = nc.dram_tensor("xtf", [P_F, KT0, B], BF16, kind="ExternalInput")
    xts_h = nc.dram_tensor("xts", [P_F, KT0, BS], BF16, kind="ExternalInput")
    xloc_h = nc.dram_tensor("xloc", [128, B], BF16, kind="ExternalInput")
    rsl_h = nc.dram_tensor("rsl", [120, nstack, FIELDS, 128], BF16, kind="ExternalInput")
    gsl_h = nc.dram_tensor("gsl", [120, nstack, F], BF16, kind="ExternalInput")
    dneg_h = nc.dram_tensor("dneg", [128, 1], BF16, kind="ExternalInput")
    hv_h = nc.dram_tensor("halves", [128, 1], BF16, kind="ExternalInput")
    w0_h = nc.dram_tensor("w0", [P_F, 2, KT0, D1], BF16, kind="ExternalInput")
    w1_h = nc.dram_tensor("w1w", [128, 8, D1], BF16, kind="ExternalInput")
    w2_h = nc.dram_tensor("w2w", [128, 4, D2], BF16, kind="ExternalInput")
    ow_h = nc.dram_tensor("outw", [128, 2, 1], BF16, kind="ExternalInput")
    wl_h = nc.dram_tensor("w1lin", [P_F, KT0, 1], BF16, kind="ExternalInput")
    b0_h = nc.dram_tensor("b0", [128, 8], F32, kind="ExternalInput")
    b1_h = nc.dram_tensor("b1", [128, 4], F32, kind="ExternalInput")
    b2_h = nc.dram_tensor("b2", [128, 2], F32, kind="ExternalInput")
    out_h = nc.dram_tensor("out", [1, BS], F32, kind="ExternalOutput")

    with tile.TileContext(nc) as tc:
        with (
            tc.tile_pool(name="persist", bufs=1) as persist,
            tc.tile_pool(name="work", bufs=2) as work,
            tc.tile_pool(name="psum", bufs=1, space="PSUM") as psum,
            tc.tile_pool(name="dram", bufs=1, space="DRAM") as dram,
        ):
            # ---------------- loads ----------------
            # Three DMA queues, each ~85GB/s — balance ~6.2MB per queue and
            # spread xtf across all three so the interaction can start early.
            # sync HW queue:   rsl -> xtf[t0:2] -> w0 half0
            # scalar HW queue: gsl -> xtf[t2:6] -> w0 half1 -> zo
            # gpsimd SW queue: xts -> xtf[t6:8] -> xloc -> smalls -> w2 -> w1
            rsl_sb = persist.tile([120, nstack, FIELDS, 128], BF16)
            nc.sync.dma_start(out=rsl_sb, in_=rsl_h[:])
            gsl_sb = persist.tile([120, nstack, F], BF16)
            nc.scalar.dma_start(out=gsl_sb, in_=gsl_h[:])
            xts_sb = persist.tile([P_F, KT0, BS], BF16)
            nc.gpsimd.dma_start(out=xts_sb, in_=xts_h[:])
            xtf_sb = persist.tile([P_F, KT0, B], BF16)
            nc.sync.dma_start(out=xtf_sb[:, 0:2, :], in_=xtf_h[:, 0:2, :])
            nc.scalar.dma_start(out=xtf_sb[:, 2:6, :], in_=xtf_h[:, 2:6, :])
            nc.gpsimd.dma_start(out=xtf_sb[:, 6:8, :], in_=xtf_h[:, 6:8, :])
            w0_sb = persist.tile([P_F, 2, KT0, D1], BF16)
            nc.sync.dma_start(out=w0_sb[:, 0, :, :], in_=w0_h[:, 0, :, :])
            nc.scalar.dma_start(out=w0_sb[:, 1, :, :], in_=w0_h[:, 1, :, :])

            xloc_sb = persist.tile([128, B], BF16)
            nc.gpsimd.dma_start(out=xloc_sb, in_=xloc_h[:])
            halves = persist.tile([128, 1], BF16)
            nc.gpsimd.dma_start(out=halves, in_=hv_h[:])
            dneg_sb = persist.tile([128, 1], BF16)
            nc.gpsimd.dma_start(out=dneg_sb, in_=dneg_h[:])
            b0_sb = persist.tile([128, 8], F32)
            nc.gpsimd.dma_start(out=b0_sb, in_=b0_h[:])
            b1_sb = persist.tile([128, 4], F32)
            nc.gpsimd.dma_start(out=b1_sb, in_=b1_h[:])
            b2_sb = persist.tile([128, 2], F32)
            nc.gpsimd.dma_start(out=b2_sb, in_=b2_h[:])
            wl_sb = persist.tile([P_F, KT0, 1], BF16)
            nc.gpsimd.dma_start(out=wl_sb, in_=wl_h[:])
            ow_sb = persist.tile([128, 2, 1], BF16)
            nc.gpsimd.dma_start(out=ow_sb, in_=ow_h[:])
            w2_sb = persist.tile([128, 4, D2], BF16)
            nc.gpsimd.dma_start(out=w2_sb, in_=w2_h[:])
            w1_sb = persist.tile([128, 8, D1], BF16)
            nc.gpsimd.dma_start(out=w1_sb, in_=w1_h[:])

            # PE warm-up (HAM clock ramp)
            warm_sb = work.tile([128, 128], BF16, tag="warm", bufs=1)
            nc.vector.memset(warm_sb, 1.0)
            ps_w = psum.tile([128, 64], F32, tag="ps_tr", bufs=1)
            for _ in range(16):
                nc.tensor.matmul(
                    ps_w, lhsT=warm_sb[:, 0:128], rhs=warm_sb[:, 0:64],
                    start=True, stop=True,
                )

            # ---------------- S build: S_loc[i in mine, j] = [128, F] ---------
            # out[i, j in J_f] = sum_{(slot,k)} R[(s,k), f, i] * G[(s,k), j]
            # (R zero-filled outside slot(i)'s K-block selects the right field
            # pair). Then PE-transpose into the [125 j, 128 i] lhsT tiles the
            # interaction needs.
            ident = work.tile([128, 128], BF16, tag="ident", bufs=1)
            make_identity(nc, ident)
            ps_sl = psum.tile([128, F], F32, tag="ps_sl", bufs=1)
            for (f, j0, j1) in fsegs:
                for st in range(nstack):
                    nc.tensor.matmul(
                        ps_sl[:, j0:j1],
                        lhsT=rsl_sb[:, st, f, :],
                        rhs=gsl_sb[:, st, j0:j1],
                        start=(st == 0),
                        stop=(st == nstack - 1),
                    )
            s_loc = persist.tile([128, F], BF16)
            nc.vector.tensor_copy(s_loc, ps_sl)
            st_sb = persist.tile([P_F, KT0, 128], BF16)
            for t in range(KT0):
                ps_tr = psum.tile([P_F, 128], BF16, tag="ps_tr", bufs=1)
                nc.tensor.transpose(
                    ps_tr, s_loc[:, t * P_F:(t + 1) * P_F], ident
                )
                nc.vector.tensor_copy(st_sb[:, t, :], ps_tr)

            # keep the HAM clock warm across the xtf DMA-wait gap
            for _ in range(12):
                nc.tensor.matmul(
                    ps_w, lhsT=warm_sb[:, 0:128], rhs=warm_sb[:, 0:64],
                    start=True, stop=True,
                )

            # ---------------- interaction partials over the FULL batch -------
            z_sb = persist.tile([1, KT0, BS], F32)
            for bc in range(KT0):
                ps_y = psum.tile([128, BS], F32, tag="ps_mm", bufs=3)
                for t in range(KT0):
                    nc.tensor.matmul(
                        ps_y,
                        lhsT=st_sb[:, t, :],
                        rhs=xtf_sb[:, t, bc * BS:(bc + 1) * BS],
                        start=(t == 0),
                        stop=(t == KT0 - 1),
                    )
                zt = work.tile([128, BS], BF16, tag="zt")
                nc.vector.tensor_mul(zt, ps_y, xloc_sb[:, bc * BS:(bc + 1) * BS])
                x2 = work.tile([128, BS], BF16, tag="x2")
                nc.vector.tensor_mul(
                    x2,
                    xloc_sb[:, bc * BS:(bc + 1) * BS],
                    xloc_sb[:, bc * BS:(bc + 1) * BS],
                )
                ps_z = psum.tile([1, BS], F32, tag="ps_z", bufs=1)
                nc.tensor.matmul(ps_z, lhsT=halves, rhs=zt, start=True, stop=False)
                nc.tensor.matmul(ps_z, lhsT=dneg_sb, rhs=x2, start=False, stop=True)
                nc.vector.tensor_copy(z_sb[:, bc, :], ps_z)

            # ---------------- z ReduceScatter (16KB fp32) --------------------
            zin = dram.tile([1, KT0, BS], F32)
            zout = dram.tile([1, BS], F32)
            nc.gpsimd.dma_start(out=zin, in_=z_sb)
            if sim_single_core:
                nc.gpsimd.dma_start(out=zout, in_=zin[:, 0, :])
            else:
                nc.gpsimd.collective_compute(
                    "ReduceScatter",
                    mybir.AluOpType.add,
                    replica_groups=[list(range(NCORES))],
                    ins=[zin[:].opt()],
                    outs=[zout[:].opt()],
                )

            # ---------------- deep MLP (batch-sharded, as v1) -----------------
            h0_sb = persist.tile([128, 8, D1], BF16)
            ps_o = psum.tile([1, BS], F32, tag="ps_o", bufs=1)
            for mj in range(8):
                ps0 = psum.tile([128, BS], F32, tag="ps_mm", bufs=3)
                for t in range(KT0):
                    nc.tensor.matmul(
                        ps0,
                        lhsT=w0_sb[:, mj // 4, t, (mj % 4) * 128:(mj % 4 + 1) * 128],
                        rhs=xts_sb[:, t, :],
                        start=(t == 0),
                        stop=(t == KT0 - 1),
                    )
                nc.scalar.activation(
                    h0_sb[:, mj, :],
                    ps0,
                    mybir.ActivationFunctionType.Relu,
                    bias=b0_sb[:, mj:mj + 1],
                )
            h1_sb = persist.tile([128, 4, BS], BF16)
            for mj in range(4):
                ps1 = psum.tile([128, BS], F32, tag="ps_mm", bufs=3)
                for t in range(8):
                    nc.tensor.matmul(
                        ps1,
                        lhsT=w1_sb[:, t, mj * 128:(mj + 1) * 128],
                        rhs=h0_sb[:, t, :],
                        start=(t == 0),
                        stop=(t == 7),
                    )
                nc.scalar.activation(
                    h1_sb[:, mj, :],
                    ps1,
                    mybir.ActivationFunctionType.Relu,
                    bias=b1_sb[:, mj:mj + 1],
                )
            h2_sb = persist.tile([128, 2, BS], BF16)
            for mj in range(2):
                ps2 = psum.tile([128, BS], F32, tag="ps_mm", bufs=3)
                for t in range(4):
                    nc.tensor.matmul(
                        ps2,
                        lhsT=w2_sb[:, t, mj * 128:(mj + 1) * 128],
                        rhs=h1_sb[:, t, :],
                        start=(t == 0),
                        stop=(t == 3),
                    )
                nc.scalar.activation(
                    h2_sb[:, mj, :],
                    ps2,
                    mybir.ActivationFunctionType.Relu,
                    bias=b2_sb[:, mj:mj + 1],
                )
            # ps_o accumulation group: deep head + linear
            for t in range(2):
                nc.tensor.matmul(
                    ps_o, lhsT=ow_sb[:, t, :], rhs=h2_sb[:, t, :],
                    start=(t == 0), stop=False,
                )
            for t in range(KT0):
                nc.tensor.matmul(
                    ps_o, lhsT=wl_sb[:, t, :], rhs=xts_sb[:, t, :],
                    start=False, stop=(t == KT0 - 1),
                )

            # ---------------- final: + RS result + folded scalar bias --------
            zo_sb = persist.tile([1, BS], F32)
            nc.scalar.dma_start(out=zo_sb, in_=zout)
            out_sb = persist.tile([1, BS], F32)
            nc.vector.tensor_scalar_add(out_sb, ps_o, float(bias_const))
            nc.vector.tensor_add(out_sb, out_sb, zo_sb)
            nc.sync.dma_start(out=out_h[:], in_=out_sb)

    nc.compile()
    return nc


def kernel(X, w1, b, nfk, f2f, deepW0, deepB0, deepW1, deepB1, deepW2, deepB2,
           outW, outB, **_unused):
    import ml_dtypes
    bf16 = ml_dtypes.bfloat16

    X = np.ascontiguousarray(X, dtype=np.float32)
    w1 = np.asarray(w1, dtype=np.float32)
    b = np.asarray(b, dtype=np.float32)
    nfk = np.ascontiguousarray(nfk, dtype=np.float32)
    f2f = np.asarray(f2f, dtype=np.int64)
    deepW0 = np.ascontiguousarray(deepW0, dtype=np.float32)
    deepW1 = np.ascontiguousarray(deepW1, dtype=np.float32)
    deepW2 = np.ascontiguousarray(deepW2, dtype=np.float32)
    outW = np.ascontiguousarray(outW, dtype=np.float32)

    # ---- host-side layout transforms (index/permutation/cast work only) ----
    perm = np.argsort(f2f, kind="stable")
    counts = np.bincount(f2f, minlength=FIELDS).astype(int)
    off = np.zeros(FIELDS + 1, dtype=np.int64)
    off[1:] = np.cumsum(counts)
    f2fs = f2f[perm]

    nslot = 6
    bins = _pack_bins(counts, nslot)
    if bins is None:
        nslot = 7
        bins = _pack_bins(counts, nslot)
    assert bins is not None, "bin packing failed"
    nstack = (nslot + 2) // 3

    # per-field global col ranges, split at PSUM bank (512 fp32) boundaries
    fsegs = []
    for f in range(FIELDS):
        c0, c1 = int(off[f]), int(off[f + 1])
        while c0 < c1:
            nxt = min(c1, (c0 // 512 + 1) * 512)
            fsegs.append((f, c0, nxt))
            c0 = nxt

    XT = np.ascontiguousarray(X[:, perm].T)                     # [F, B]
    nfkp = nfk[perm]                                            # [F, FIELDS, K]
    W0p = np.ascontiguousarray(deepW0[perm])
    w1p = np.ascontiguousarray(w1[perm].reshape(F, 1))
    bias_const = float(np.float32(b[0]) + np.float32(outB[0]))

    nc = _build_program(fsegs, nstack, bias_const)

    def _c(a, dt=bf16):
        return np.ascontiguousarray(a).astype(dt)

    xtf_dev = _c(XT.reshape(KT0, P_F, B).transpose(1, 0, 2))
    w0_dev = _c(W0p.reshape(KT0, P_F, 2, D1).transpose(1, 2, 0, 3))
    w1_dev = _c(deepW1.reshape(8, 128, D1).transpose(1, 0, 2))
    w2_dev = _c(deepW2.reshape(4, 128, D2).transpose(1, 0, 2))
    ow_dev = _c(outW.reshape(2, 128, 1).transpose(1, 0, 2))
    wl_dev = _c(w1p.reshape(KT0, P_F, 1).transpose(1, 0, 2))
    b0_dev = np.ascontiguousarray(np.asarray(deepB0, np.float32).reshape(8, 128).T)
    b1_dev = np.ascontiguousarray(np.asarray(deepB1, np.float32).reshape(4, 128).T)
    b2_dev = np.ascontiguousarray(np.asarray(deepB2, np.float32).reshape(2, 128).T)
    halves_dev = np.full((128, 1), 0.5, dtype=bf16)

    in_maps = []
    for c in range(NCORES):
        groups = bins[c]
        myrows = np.concatenate(
            [np.arange(off[g], off[g + 1]) for g in groups]
        ).astype(np.int64)
        slot_of = np.concatenate(
            [np.full(counts[g], s, np.int64) for s, g in enumerate(groups)]
        )
        nmy = len(myrows)
        # R: [nstack, 120, FIELDS, 128] zero-padded slot encoding
        R = np.zeros((nstack, 120, FIELDS, 128), np.float32)
        for li in range(nmy):
            gi, s = myrows[li], slot_of[li]
            stk, sl = divmod(int(s), 3)
            R[stk, sl * K:(sl + 1) * K, :, li] = nfkp[gi].T
        G = np.zeros((nstack, 120, F), np.float32)
        for s, g in enumerate(groups):
            stk, sl = divmod(int(s), 3)
            G[stk, sl * K:(sl + 1) * K, :] = nfkp[:, g, :].T
        xloc = np.zeros((128, B), np.float32)
        xloc[:nmy] = XT[myrows]
        dvec = np.zeros((128, 1), np.float32)
        dvec[:nmy, 0] = np.sum(nfkp[myrows, f2fs[myrows], :] ** 2, axis=1)
        in_maps.append({
            "xtf": xtf_dev,
            "xts": _c(XT[:, c * BS:(c + 1) * BS].reshape(KT0, P_F, BS).transpose(1, 0, 2)),
            "xloc": _c(xloc),
            "rsl": _c(R.transpose(1, 0, 2, 3)),
            "gsl": _c(G.transpose(1, 0, 2)),
            "dneg": _c(-0.5 * dvec),
            "halves": halves_dev,
            "w0": w0_dev,
            "w1w": w1_dev,
            "w2w": w2_dev,
            "outw": ow_dev,
            "w1lin": wl_dev,
            "b0": b0_dev, "b1": b1_dev, "b2": b2_dev,
        })

    res = run_bass_kernel_spmd(nc, in_maps, core_ids=list(range(NCORES)))
    global LAST_RESULT
    LAST_RESULT = res
    out = np.concatenate([r["out"].reshape(-1) for r in res.results])
    return out.astype(np.float32)


LAST_RESULT = None


if __name__ == "__main__":
    import importlib.util as _iu

    spec = _iu.spec_from_file_location("ref", "/root/problem/reference.py")
    ref = _iu.module_from_spec(spec)
    spec.loader.exec_module(ref)
    inp = {k: np.asarray(v) for k, v in ref.setup_inputs().items()}
    got = kernel(**inp)
    print("kernel out:", got[:8])
